# revision 21
# baseline (speedup 1.0000x reference)
"""Trainium2 Bass kernel for a StyleGAN-style modulated conv2d.

Reference math (see problem statement):
    w  = kernel * he_std                       # equalized-lr
    s  = style @ w_mod + b_mod + 1             # [B, cin]
    s  = s / max|s|                            # global max-abs over [B, cin]
    w  = w * s[0][None, None, :, None]         # style[0] only -> one shared weight
    d  = rsqrt(sum(w^2, (0,1,2)) + 1e-8)
    w  = w * d
    y  = conv2d_same(x, w) + noise*(ns/2) + bias
    y  = lrelu(y, 0.2) * sqrt(2)

Because only style[0] modulates, the effective 3x3x128x128 weight is identical
for every batch element, so the device work is a plain 3x3 conv. The tiny
modulation math (a 512x128 matvec + norms, ~1e-6 of total FLOPs) is folded on
the host while sharding; the conv + activation run on 8 NeuronCores,
data-parallel over batch (1 image per core).

Device strategy per core:
  - x is pre-padded/transposed on the host to [cin=128, 258, 258] bf16 (zero
    SAME-padding baked in), so every DMA is a plain linear per-partition copy.
  - 3x3 conv = 9 accumulating matmuls per PSUM group: lhsT = w[cin,cout] per
    tap, rhs = shifted x rows ([2 rows x 256 cols] = 512 spatial AP), PSUM
    [cout=128, 512] fp32. Stream floor is 9*128 matmuls x ~216ns = ~249us.
  - sqrt(2) (the lrelu gain) and the demod factor d are folded into the
    weights on the host, so the epilogue is a single fused DVE op per group:
        y = max(0.2*z, z)  ==  lrelu(z, 0.2)        (z = sqrt2 * conv)
    via scalar_tensor_tensor(out, ps, 0.2, ps, mult, max), writing bf16.
  - HAM warm-up: the PE clock sits at 1.2 GHz until ~3.4us of sustained
    matmul activity. A handful of dummy matmuls issued at t=0 (during the
    first input DMA) warm it so the real stream runs at 2.4 GHz from the
    first group. The first slab's DMA is split so real matmuls start ~3us in.
  - Output stays [cout, H*W] bf16 per core; host upcasts + transposes to NHWC.
"""

import math
from contextlib import ExitStack

import ml_dtypes
import numpy as np

import concourse.bacc as bacc
import concourse.bass as bass
import concourse.mybir as mybir
import concourse.tile as tile
from concourse.bass_utils import run_bass_kernel_spmd

B, H, W, CIN, COUT, KK, SDIM = 8, 256, 256, 128, 128, 3, 512
HP, WP = H + 2, W + 2  # zero-padded spatial dims (SAME padding for 3x3)
N_CORES = 8
ROWS_PER_SLAB = 32          # output rows per input slab
SLABS = H // ROWS_PER_SLAB  # 8
GROUP_ROWS = 2              # output rows per PSUM group (2*256 = 512 = 1 bank)
OUT_TILE_ROWS = 8           # rows per SBUF output tile (8*256*2B = 4KB/part)
N_WARMUP_MM = 34            # dummy matmuls bridging the first-chunk DMA wait

BF16 = mybir.dt.bfloat16
F32 = mybir.dt.float32
SQRT2 = float(np.sqrt(np.float32(2.0)))


def _effective_weight(style, kernel, w_mod, b_mod):
    """Exactly the reference weight math, in fp32 numpy, times sqrt(2).

    The sqrt(2) lrelu gain commutes with the conv, so folding it here turns
    the device epilogue into a pure lrelu: sqrt2*lrelu(z,.2) = lrelu(sqrt2*z,.2).
    """
    style = np.asarray(style, np.float32)
    kernel = np.asarray(kernel, np.float32)
    w_mod = np.asarray(w_mod, np.float32)
    b_mod = np.asarray(b_mod, np.float32)

    he_std = np.float32(1.0) / np.sqrt(np.float32(KK * KK * CIN))
    w = kernel * he_std
    s = (style @ w_mod + b_mod + np.float32(1.0)).astype(np.float32)
    s = s * (np.float32(1.0) / np.max(np.abs(s)))
    w = w * s[0][None, None, :, None]
    d = np.float32(1.0) / np.sqrt(
        np.sum(np.square(w), axis=(0, 1, 2), dtype=np.float32) + np.float32(1e-8)
    )
    w = w * d[None, None, None, :]
    return (w * np.float32(SQRT2)).astype(np.float32)  # [3, 3, cin, cout]


def _build_program(with_noise: bool, with_bias: bool):
    # Bacc (not raw Bass): its compile() splits multi-sem sync waits into
    # event semaphores — TRN2 allows at most one wait per instruction.
    nc = bacc.Bacc(trn_type="TRN2")
    x = nc.declare_dram_parameter("x", [CIN, HP * WP], BF16, isOutput=False)
    w = nc.declare_dram_parameter("w", [CIN, 9 * COUT], BF16, isOutput=False)
    if with_bias:
        ab = nc.declare_dram_parameter("ab", [COUT, 1], F32, isOutput=False)
    if with_noise:
        nz = nc.declare_dram_parameter("nz", [1, H * W], BF16, isOutput=False)
        ones = nc.declare_dram_parameter("ones", [1, COUT], BF16, isOutput=False)
    y = nc.declare_dram_parameter("y", [COUT, H * W], BF16, isOutput=True)

    slab_rows_in = ROWS_PER_SLAB + 2  # input halo rows per slab
    mult, amax = mybir.AluOpType.mult, mybir.AluOpType.max

    with ExitStack() as ctx:
        tc = ctx.enter_context(tile.TileContext(nc))
        consts = ctx.enter_context(tc.tile_pool(name="consts", bufs=1))
        warm = ctx.enter_context(tc.tile_pool(name="warm", bufs=1))
        warmps = ctx.enter_context(tc.tile_pool(name="warmps", bufs=1, space="PSUM"))
        xpool = ctx.enter_context(tc.tile_pool(name="x", bufs=3))
        opool = ctx.enter_context(tc.tile_pool(name="out", bufs=3))
        pspool = ctx.enter_context(tc.tile_pool(name="ps", bufs=3, space="PSUM"))
        tpool = ctx.enter_context(tc.tile_pool(name="tmp", bufs=4))
        if with_noise:
            nzpool = ctx.enter_context(tc.tile_pool(name="nz", bufs=2))

        # HAM warm-up: PE-busy dummy matmuls issued while the first input
        # DMA is in flight, so the PE clock is at 2.4GHz when real work
        # arrives. Results land in a scratch PSUM bank, never read. N=128
        # keeps each one ~107ns (cold) so the real stream slots in with
        # minimal queueing delay once its input chunk lands.
        dmy = warm.tile([CIN, COUT], BF16)
        nc.vector.memset(dmy[:], 0.0)
        dps = warmps.tile([COUT, COUT], F32)
        for _ in range(N_WARMUP_MM):
            nc.tensor.matmul(dps[:], dmy[:], dmy[:], start=True, stop=True)

        # All DMAs on HWDGE (nc.sync): a gpsimd (SWDGE) DMA costs a one-time
        # ~5us Q7 library load at first use, which delays the weights enough
        # that the HAM re-throttles the PE (measured: +4us end-to-end).
        wt = consts.tile([CIN, 9 * COUT], BF16)
        nc.sync.dma_start(wt[:], w[:])
        if with_bias:
            abt = consts.tile([COUT, 1], F32)
            nc.sync.dma_start(abt[:], ab[:])
        if with_noise:
            onest = consts.tile([1, COUT], BF16)
            nc.sync.dma_start(onest[:], ones[:])

        # Concurrent DMAs share HBM bandwidth round-robin per packet, so a
        # prefetch issued at t=0 starves the latency-critical first rows
        # (baseline trace: first matmul at ~18us). Startup DMAs are therefore
        # serialized with dependency gates: a 1-element DVE copy from a tile
        # the trigger group's epilogue wrote into the target tile makes the
        # following dma_start wait (WAW) until that group is done.
        xtiles = {}

        def chunk_dma(slab, lo, hi, gate_src=None):
            xt = xtiles[slab]
            r0 = slab * ROWS_PER_SLAB
            if gate_src is not None:
                nc.vector.tensor_copy(
                    xt[0:1, lo * WP : lo * WP + 1], gate_src[0:1, 0:1]
                )
            nc.sync.dma_start(
                xt[:, lo * WP : hi * WP],
                x[:, (r0 + lo) * WP : (r0 + hi) * WP],
            )

        # (slab, pair-in-slab) -> DMA to emit after that pair's epilogue.
        # Pair p computes output rows 4p..4p+3, reading input rows 4p..4p+5.
        # A1 (rows 0..5, small => lands fastest) unblocks pair 0; A2 (..13)
        # is chained behind A1 on the same ring; B/C/slab prefetches hang
        # off epilogues so they never steal bandwidth from earlier chunks.
        triggers = {
            (0, 0): (0, 14, 22),
            (0, 1): (0, 22, slab_rows_in),
            (0, 3): (1, 0, slab_rows_in),
            (0, 6): (2, 0, slab_rows_in),
        }
        xtiles[0] = xpool.tile([CIN, slab_rows_in * WP], BF16, name="xt0", tag="xt")
        chunk_dma(0, 0, 6)
        chunk_dma(0, 6, 14, gate_src=xtiles[0][:, 0:1])  # chain A2 after A1

        for slab in range(SLABS):
            r0 = slab * ROWS_PER_SLAB  # first output row of the slab
            if slab >= 3:
                # steady state: tile recycling (bufs=3) already gates these
                xtiles[slab] = xpool.tile(
                    [CIN, slab_rows_in * WP], BF16, name=f"xt{slab}", tag="xt"
                )
                chunk_dma(slab, 0, slab_rows_in)
            xv = xtiles[slab][:].rearrange("p (r c) -> p r c", c=WP)
            if with_noise:
                nzt = nzpool.tile([1, ROWS_PER_SLAB * W], BF16)
                nc.sync.dma_start(nzt[:], nz[:, r0 * W : (r0 + ROWS_PER_SLAB) * W])

            for half in range(ROWS_PER_SLAB // OUT_TILE_ROWS):
                ot = opool.tile([COUT, OUT_TILE_ROWS * W], BF16)
                last_tile = slab == SLABS - 1 and half == ROWS_PER_SLAB // OUT_TILE_ROWS - 1
                base = half * OUT_TILE_ROWS
                # A "job" = one PSUM tile + one epilogue. Normal jobs pair two
                # 2-row matmul chunks in one 2-bank PSUM tile, halving epilogue
                # instructions and cross-engine sync edges. The last tile ends
                # in small solo jobs so the final epilogue+DMA chain after the
                # very last matmul is as short as possible.
                if last_tile:
                    jobs = [[(base, 2)], [(base + 2, 2)], [(base + 4, 2)],
                            [(base + 6, 1)], [(base + 7, 1)]]
                else:
                    jobs = [[(base, 2), (base + 2, 2)],
                            [(base + 4, 2), (base + 6, 2)]]
                for j, chunks in enumerate(jobs):
                    rr0 = chunks[0][0]
                    tot = sum(nr for _, nr in chunks)
                    ps = pspool.tile([COUT, tot * W], F32, name="ps", tag="ps")
                    off = 0
                    for rr, nr in chunks:
                        psv = ps[:, off * W : (off + nr) * W]
                        for t in range(9):
                            dh, dw = divmod(t, 3)
                            rhs = xv[:, rr + dh : rr + dh + nr, dw : dw + W]
                            nc.tensor.matmul(
                                psv,
                                wt[:, t * COUT : (t + 1) * COUT],
                                rhs,
                                start=(t == 0),
                                stop=(t == 8 and not with_noise),
                            )
                        if with_noise:
                            nc.tensor.matmul(
                                psv,
                                onest[:],
                                nzt[:, rr * W : (rr + nr) * W],
                                start=False,
                                stop=True,
                            )
                        off += nr
                    # Epilogue: weights carry sqrt2*demod, so
                    # y = lrelu(z+b, 0.2) = max(0.2*t, t), t = z + b.
                    # ACT evacuates PSUM (+bias, free in its affine stage),
                    # one fused DVE op does the lrelu — balanced engines.
                    oslice = ot[:, (rr0 - base) * W : (rr0 - base + tot) * W]
                    t1 = tpool.tile([COUT, tot * W], F32, name="t1", tag="t1")
                    nc.scalar.activation(
                        t1[:],
                        ps[:],
                        mybir.ActivationFunctionType.Identity,
                        bias=abt[:, 0:1] if with_bias else 0.0,
                        scale=1.0,
                    )
                    nc.vector.scalar_tensor_tensor(
                        oslice, t1[:], 0.2, t1[:], mult, amax
                    )
                    trig = triggers.get((slab, half * 2 + j))
                    if trig is not None:
                        tslab, lo, hi = trig
                        if tslab not in xtiles:
                            xtiles[tslab] = xpool.tile(
                                [CIN, slab_rows_in * WP], BF16, name=f"xt{tslab}", tag="xt"
                            )
                        chunk_dma(tslab, lo, hi, gate_src=oslice)
                    if last_tile:
                        # Drain the final tile per job so the tail after
                        # the last matmul is one small DMA, not a 0.5MB one.
                        row = r0 + rr0
                        nc.sync.dma_start(
                            y[:, row * W : (row + tot) * W], oslice
                        )
                if not last_tile:
                    row = r0 + half * OUT_TILE_ROWS
                    nc.sync.dma_start(
                        y[:, row * W : (row + OUT_TILE_ROWS) * W], ot[:]
                    )
    nc.finalize()  # Bacc.compile(): reg alloc + split multi-sem waits (TRN2)
    return nc


def _run(inputs, trace=False, **spmd_kwargs):
    x = np.asarray(inputs["x"])
    noise_strength = float(np.asarray(inputs["noise_strength"]).reshape(-1)[0])
    bias = np.asarray(inputs["bias"], np.float32)

    w_eff = _effective_weight(
        inputs["style"], inputs["kernel"], inputs["w_mod"], inputs["b_mod"]
    )
    # [3,3,cin,cout] -> [cin, tap*cout], tap-major free dim
    w_dev = np.ascontiguousarray(
        w_eff.transpose(2, 0, 1, 3).reshape(CIN, 9 * COUT)
    ).astype(ml_dtypes.bfloat16)

    # Pad + NHWC->NCHW per image, cast bf16. Zero borders bake in SAME padding.
    x_pad = np.zeros((B, CIN, HP, WP), dtype=ml_dtypes.bfloat16)
    x_pad[:, :, 1 : H + 1, 1 : W + 1] = x.transpose(0, 3, 1, 2).astype(
        ml_dtypes.bfloat16
    )

    with_noise = noise_strength != 0.0
    with_bias = bool(np.any(bias != 0.0))
    in_maps = []
    for b in range(B):
        m = {
            "x": np.ascontiguousarray(x_pad[b].reshape(CIN, HP * WP)),
            "w": w_dev,
        }
        if with_bias:
            # sqrt2 folded to match the sqrt2-scaled conv output
            m["ab"] = (bias * np.float32(SQRT2)).reshape(COUT, 1).astype(np.float32)
        if with_noise:
            nzb = np.asarray(inputs["noise"], np.float32)[b, :, :, 0] * np.float32(
                noise_strength / 2.0 * SQRT2
            )
            m["nz"] = nzb.reshape(1, H * W).astype(ml_dtypes.bfloat16)
            m["ones"] = np.ones((1, COUT), dtype=ml_dtypes.bfloat16)
        in_maps.append(m)

    nc = _build_program(with_noise, with_bias)
    res = run_bass_kernel_spmd(
        nc, in_maps, list(range(N_CORES)), trace=trace, **spmd_kwargs
    )

    out = np.empty((B, H, W, COUT), dtype=np.float32)
    for b in range(B):
        out[b] = (
            res.results[b]["y"].astype(np.float32).reshape(COUT, H, W).transpose(1, 2, 0)
        )
    return out, res


def kernel(**inputs):
    out, _ = _run(inputs)
    return out
